# revision 6
# baseline (speedup 1.0000x reference)
"""RNN-T greedy-decoder state update, batch-sharded over 8 NeuronCores.

Strategy (pure data parallel over batch B=1024 -> 128 rows/core = 128 SBUF
partitions):
  * One combined [128,532] i32 load carries res (0:256), a zeroed slot for the
    4 int outputs (256:260), the per-batch int vectors + row-index constants
    (260:276), and the column-index iota row (276:532) -- so no POOL-engine
    iota op and a single HWDGE load.
  * Every `where(not_blank, a, b)` over a large tensor is an indirect-DMA row
    gather from DRAM, so only the selected rows ever move:
      - fi_new: one gather from [f_shard(128*128 rows); fi(128 rows)] x 1024
      - (pre_hg,pre_cg)_new: one gather from a host-packed [256, 4*320] tensor
        whose row sel*128+p holds [x[0,b], x[1,b] for x in (h, c)]
  * The transcript scatter (res) is a compare mask + copy_predicated in the
    combined tile; one [128,260] store covers res_new and the int outputs.
  * done-flag is reduced on host from the gathered time_idx_new/f_lens
    (the batch-sharded equivalent of the all-reduce AND).
"""

import os

import numpy as np

B, T, H_ENC = 1024, 128, 1024
L, H_PRED = 2, 320
MAXLEN = 256
BLANK = 28
MAX_SYMBOLS = 30
N_CORES = 8
BC = B // N_CORES  # 128 batch rows per core == SBUF partitions
HC_W = 2 * L * H_PRED  # 1280: [h_l0 | h_l1 | c_l0 | c_l1]
CIN_W = MAXLEN + 4 + 16 + MAXLEN  # 532

_CACHE = {}


def _build_nc():
    import concourse.bass as bass
    import concourse.mybir as mybir
    from concourse.tile import TileContext

    i32 = mybir.dt.int32
    f32 = mybir.dt.float32
    Alu = mybir.AluOpType

    nc = bass.Bass()

    cin = nc.declare_dram_parameter("cin", [BC, CIN_W], i32, isOutput=False)
    fext = nc.declare_dram_parameter("fext", [BC * T + BC, H_ENC], f32, isOutput=False)
    hcall = nc.declare_dram_parameter("hcall", [2 * BC, HC_W], f32, isOutput=False)

    cres_o = nc.declare_dram_parameter("cres_o", [BC, MAXLEN + 4], i32, isOutput=True)
    fi_o = nc.declare_dram_parameter("fi_o", [BC, H_ENC], f32, isOutput=True)
    hc_o = nc.declare_dram_parameter("hc_o", [BC, HC_W], f32, isOutput=True)

    with TileContext(nc) as tc:
        with tc.tile_pool(name="pool", bufs=1) as pool:
            ct = pool.tile([BC, CIN_W], i32)
            nc.sync.dma_start(out=ct[:], in_=cin[:])

            sym = ct[:, 260:261]
            sa = ct[:, 261:262]
            ri = ct[:, 262:263]
            fl = ct[:, 263:264]
            ti = ct[:, 264:265]
            pg = ct[:, 265:266]
            pidx = ct[:, 266:267]    # p
            frowb = ct[:, 267:268]   # p*T
            firow = ct[:, 268:269]   # T*BC + p
            col = ct[:, 276:532]     # iota row 0..255

            # not_blank = (sym != BLANK) & (sa < MAX_SYMBOLS) & (ti < fl)
            t0 = pool.tile([BC, 1], i32)
            t1 = pool.tile([BC, 1], i32)
            t2 = pool.tile([BC, 1], i32)
            nb = pool.tile([BC, 1], i32)
            nc.vector.tensor_scalar(t0[:], sym, BLANK, None, Alu.not_equal)
            nc.vector.tensor_scalar(t1[:], sa, MAX_SYMBOLS, None, Alu.is_lt)
            nc.vector.tensor_tensor(t2[:], ti, fl, Alu.is_lt)
            nc.vector.tensor_tensor(t0[:], t0[:], t1[:], Alu.mult)
            nc.vector.tensor_tensor(nb[:], t0[:], t2[:], Alu.mult)

            # combined output tile (DVE is its only writer, so the store
            # needs a single sem wait -- HWDGE DMAs only support one)
            co_t = pool.tile([BC, MAXLEN + 4], i32)
            # int outputs into co_t cols 256..260:
            # 256: symbols_added_new = (sa + 1) * nb
            # 257: res_idx_new       = ri + nb
            # 258: time_idx_new      = ti + 1 - nb
            # 259: pre_g_new         = nb ? sym : pg
            tmp = pool.tile([BC, 1], i32)
            nc.vector.tensor_scalar(tmp[:], sa, 1, None, Alu.add)
            nc.vector.tensor_tensor(co_t[:, 256:257], tmp[:], nb[:], Alu.mult)
            nc.vector.tensor_tensor(co_t[:, 257:258], ri, nb[:], Alu.add)
            tmp2 = pool.tile([BC, 1], i32)
            nc.vector.tensor_scalar(tmp2[:], ti, 1, None, Alu.add)
            nc.vector.tensor_tensor(co_t[:, 258:259], tmp2[:], nb[:], Alu.subtract)
            nc.vector.tensor_copy(co_t[:, 259:260], pg)
            nc.vector.copy_predicated(co_t[:, 259:260], nb[:], sym)

            # f row index: nb ? (T*BC + p) : (p*T + min(time_idx_new, T-1))
            tg = pool.tile([BC, 1], i32)
            nc.vector.tensor_scalar(tg[:], co_t[:, 258:259], T - 1, None, Alu.min)
            frow = pool.tile([BC, 1], i32)
            nc.vector.tensor_tensor(frow[:], tg[:], frowb, Alu.add)
            nc.vector.copy_predicated(frow[:], nb[:], firow)

            # state row index: nb*BC + p
            hsel = pool.tile([BC, 1], i32)
            nc.vector.tensor_scalar(hsel[:], nb[:], BC, None, Alu.mult)
            hrow = pool.tile([BC, 1], i32)
            nc.vector.tensor_tensor(hrow[:], hsel[:], pidx, Alu.add)

            # gathers: only the selected rows are read from HBM
            fi_t = pool.tile([BC, H_ENC], f32)
            nc.gpsimd.indirect_dma_start(
                out=fi_t[:], out_offset=None, in_=fext[:],
                in_offset=bass.IndirectOffsetOnAxis(ap=frow[:, :1], axis=0),
            )
            hc_t = pool.tile([BC, HC_W], f32)
            nc.gpsimd.indirect_dma_start(
                out=hc_t[:], out_offset=None, in_=hcall[:],
                in_offset=bass.IndirectOffsetOnAxis(ap=hrow[:, :1], axis=0),
            )

            # transcript scatter: res[b, res_idx[b]] = symbols[b] where nb
            meq = pool.tile([BC, MAXLEN], i32)
            nc.vector.tensor_tensor(meq[:], col, ri.to_broadcast([BC, MAXLEN]), Alu.is_equal)
            nc.vector.tensor_tensor(meq[:], meq[:], nb[:, :1].to_broadcast([BC, MAXLEN]), Alu.mult)
            nc.vector.tensor_copy(co_t[:, 0:MAXLEN], ct[:, 0:MAXLEN])
            nc.vector.copy_predicated(co_t[:, 0:MAXLEN], meq[:], sym.to_broadcast([BC, MAXLEN]))

            # stores
            nc.sync.dma_start(out=cres_o[:], in_=co_t[:])
            nc.sync.dma_start(out=fi_o[:], in_=fi_t[:])
            nc.sync.dma_start(out=hc_o[:], in_=hc_t[:])

    _split_multiwait(nc, mybir)
    return nc


def _split_multiwait(nc, mybir):
    """walrus on this image only encodes 1 sync wait per instruction; split
    extra waits into single-wait NOPs appended to the previous block (same
    engine, so they execute right before the multi-wait instruction)."""
    f = nc.m.functions[0]
    blocks = f.blocks
    for bi, bb in enumerate(blocks):
        for inst in bb.instructions:
            si = inst.sync_info
            if not si or len(si.on_wait) <= 1:
                continue
            waits = list(si.on_wait)
            assert bi > 0, "multi-wait in first block unsupported"
            prev_bb = blocks[bi - 1]
            for k, w in enumerate(waits[:-1]):
                nop = mybir.InstNoOp(name=f"{inst.name}-presync{k}", ins=[], outs=[])
                nop.engine = inst.engine
                nop.sync_info = mybir.SyncInfo(on_wait=[w], on_update=[])
                prev_bb.add_instruction(nop)
            inst.sync_info = mybir.SyncInfo(
                on_wait=[waits[-1]], on_update=list(si.on_update)
            )


def _get_nc():
    if "nc" not in _CACHE:
        _CACHE["nc"] = _build_nc()
    return _CACHE["nc"]


def _make_in_maps(symbols, symbols_added, res, res_idx, f, f_lens, time_idx,
                  fi, pre_g, pre_hg, pre_cg, hg, cg):
    p = np.arange(BC, dtype=np.int32)
    in_maps = []
    for c in range(N_CORES):
        sl = slice(c * BC, (c + 1) * BC)
        cin = np.zeros((BC, CIN_W), np.int32)
        cin[:, 0:MAXLEN] = res[sl]
        cin[:, 260] = symbols[sl]
        cin[:, 261] = symbols_added[sl]
        cin[:, 262] = res_idx[sl]
        cin[:, 263] = f_lens[sl]
        cin[:, 264] = time_idx[sl]
        cin[:, 265] = pre_g[sl]
        cin[:, 266] = p
        cin[:, 267] = p * T
        cin[:, 268] = T * BC + p
        cin[:, 276:532] = np.arange(MAXLEN, dtype=np.int32)[None, :]
        fext = np.concatenate([f[sl].reshape(BC * T, H_ENC), fi[sl]], axis=0)
        # hcall row p      = [pre_hg[0,b], pre_hg[1,b], pre_cg[0,b], pre_cg[1,b]]
        # hcall row BC + p = [   hg[0,b],    hg[1,b],    cg[0,b],    cg[1,b]]
        keep = np.concatenate(
            [pre_hg[:, sl].transpose(1, 0, 2).reshape(BC, L * H_PRED),
             pre_cg[:, sl].transpose(1, 0, 2).reshape(BC, L * H_PRED)], axis=1
        )
        take = np.concatenate(
            [hg[:, sl].transpose(1, 0, 2).reshape(BC, L * H_PRED),
             cg[:, sl].transpose(1, 0, 2).reshape(BC, L * H_PRED)], axis=1
        )
        hcall = np.concatenate([keep, take], axis=0)
        in_maps.append({"cin": cin, "fext": fext, "hcall": hcall})
    return in_maps


def kernel(**inputs):
    from concourse.bass_utils import run_bass_kernel_spmd

    symbols = np.asarray(inputs["symbols"], dtype=np.int32)
    symbols_added = np.asarray(inputs["symbols_added"], dtype=np.int32)
    res = np.asarray(inputs["res"], dtype=np.int32)
    res_idx = np.asarray(inputs["res_idx"], dtype=np.int32)
    f = np.asarray(inputs["f"], dtype=np.float32)
    f_lens = np.asarray(inputs["f_lens"], dtype=np.int32)
    time_idx = np.asarray(inputs["time_idx"], dtype=np.int32)
    fi = np.asarray(inputs["fi"], dtype=np.float32)
    pre_g = np.asarray(inputs["pre_g"], dtype=np.int32)
    pre_hg = np.asarray(inputs["pre_hg"], dtype=np.float32)
    pre_cg = np.asarray(inputs["pre_cg"], dtype=np.float32)
    hg = np.asarray(inputs["hg"], dtype=np.float32)
    cg = np.asarray(inputs["cg"], dtype=np.float32)

    in_maps = _make_in_maps(symbols, symbols_added, res, res_idx, f, f_lens,
                            time_idx, fi, pre_g, pre_hg, pre_cg, hg, cg)

    nc = _get_nc()
    trace = bool(int(os.environ.get("KERNEL_TRACE", "0")))
    r = run_bass_kernel_spmd(nc, in_maps, list(range(N_CORES)), trace=trace)
    _CACHE["last_results"] = r
    outs = r.results

    cres = np.concatenate([outs[c]["cres_o"] for c in range(N_CORES)], axis=0)
    res_new = np.ascontiguousarray(cres[:, :MAXLEN])
    symbols_added_new = cres[:, 256].copy()
    res_idx_new = cres[:, 257].copy()
    time_idx_new = cres[:, 258].copy()
    pre_g_new = cres[:, 259].copy()
    fi_new = np.concatenate([outs[c]["fi_o"] for c in range(N_CORES)], axis=0)
    hc = np.stack([outs[c]["hc_o"] for c in range(N_CORES)])  # [C, BC, 1280]
    hc = hc.reshape(N_CORES, BC, 2, L, H_PRED)  # [C, BC, (h|c), L, H]
    pre_hg_new = np.ascontiguousarray(
        hc[:, :, 0].transpose(2, 0, 1, 3).reshape(L, B, H_PRED)
    )
    pre_cg_new = np.ascontiguousarray(
        hc[:, :, 1].transpose(2, 0, 1, 3).reshape(L, B, H_PRED)
    )
    done = np.all(time_idx_new >= f_lens)

    return (
        symbols_added_new,
        res_new,
        res_idx_new,
        time_idx_new,
        fi_new,
        pre_g_new,
        pre_hg_new,
        pre_cg_new,
        done,
    )
